# revision 1
# baseline (speedup 1.0000x reference)
"""GroupedQueryAttention TRN2 Bass kernel, sharded over 8 NeuronCores.

Problem (hardcoded): B=2, T=2048, D=4096, 32 Q heads x 128, 8 KV groups x 128,
RoPE (base 5e5), causal, out = ctx @ Wo.

Sharding: core g owns Q heads 4g..4g+3 (Wq columns 512g:512g+512), KV group g
(Wk/Wv columns 128g:128g+128), and Wo rows 512g:512g+512 (row-parallel).
Each core computes a full-shape partial output; host sums the 8 partials.

All matmuls run in float32r (fp32 with 11-bit mantissa, full PE rate).
Inputs are pre-rounded to fp32r on host; on-device producers write f32r.
"""
import sys
import numpy as np

for _p in ("/opt/trn_rl_repo", "/root/.axon_site", "/root/.axon_site/_ro/trn_rl_repo"):
    if _p not in sys.path:
        sys.path.append(_p)

from contextlib import ExitStack

import concourse.bass as bass
import concourse.tile as tile
from concourse import bacc, mybir
from concourse.bass_utils import run_bass_kernel_spmd
from concourse.masks import make_identity

B, T, D = 2, 2048, 4096
NH, NKV, DH = 32, 8, 128
HPC = NH // 8          # 4 q heads per core
FPC = HPC * DH         # 512 q features per core
ROPE_BASE = 500000.0
NT = B * T             # 4096 tokens
f32 = mybir.dt.float32
f32r = mybir.dt.float32r
EXP_SCALE = 1.0 / float(np.sqrt(DH))

_NC_CACHE = {}


def _round_fp32r(x):
    x = np.ascontiguousarray(x, dtype=np.float32)
    u = x.view(np.uint32)
    lsb = (u >> 12) & np.uint32(1)
    r = (u + np.uint32(0x7FF) + lsb) & np.uint32(0xFFFFF000)
    return r.view(np.float32)


def _build_program():
    nc = bacc.Bacc("TRN2", target_bir_lowering=False, debug=False)

    xT = nc.dram_tensor("xT", [D, NT], f32r, kind="ExternalInput").ap()
    wq = nc.dram_tensor("wq", [D, FPC], f32r, kind="ExternalInput").ap()
    wk = nc.dram_tensor("wk", [D, DH], f32r, kind="ExternalInput").ap()
    wv = nc.dram_tensor("wv", [D, DH], f32r, kind="ExternalInput").ap()
    wo = nc.dram_tensor("wo", [FPC, D], f32r, kind="ExternalInput").ap()
    ropeA = nc.dram_tensor("ropeA", [128, T], f32, kind="ExternalInput").ap()
    ropeB = nc.dram_tensor("ropeB", [128, T], f32, kind="ExternalInput").ap()
    masks = nc.dram_tensor("masks", [512, 512], f32r, kind="ExternalInput").ap()
    outp = nc.dram_tensor("outp", [NT, D], f32, kind="ExternalOutput").ap()

    qTd = nc.dram_tensor("qTd", [FPC, NT], f32r).ap()     # roped Q^T, feature-major
    ctxd = nc.dram_tensor("ctxd", [FPC, NT], f32r).ap()   # attention out^T

    KC = D // 128  # 32 contraction chunks

    with tile.TileContext(nc) as tc, ExitStack() as s0:
        kvp = s0.enter_context(tc.tile_pool(name="kv", bufs=1))
        KTb = [kvp.tile([128, T], f32r, tag=f"KT{i}", name=f"KT{i}") for i in range(B)]
        Vb = [kvp.tile([128, T], f32r, tag=f"V{i}", name=f"V{i}") for i in range(B)]
        ident_f = kvp.tile([128, 128], f32, tag="ident_f")
        make_identity(nc, ident_f[:])
        ident = kvp.tile([128, 128], f32r, tag="ident")
        nc.vector.tensor_copy(ident[:], ident_f[:])
        ones_f = kvp.tile([128, 1], f32, tag="ones_f")
        nc.vector.memset(ones_f[:], 1.0)
        ones = kvp.tile([128, 1], f32r, tag="ones")
        nc.vector.tensor_copy(ones[:], ones_f[:])
        ones_row_f = kvp.tile([1, 128], f32, tag="ones_row_f")
        nc.vector.memset(ones_row_f[:], 1.0)
        ones_row = kvp.tile([1, 128], f32r, tag="ones_row")
        nc.vector.tensor_copy(ones_row[:], ones_row_f[:])

        # ---------------- Phase A: projections + RoPE -----------------
        with ExitStack() as sa:
            wp = sa.enter_context(tc.tile_pool(name="wts", bufs=1))
            wq_sb = wp.tile([128, KC * FPC], f32r, tag="wq")
            wk_sb = wp.tile([128, KC * DH], f32r, tag="wk")
            wv_sb = wp.tile([128, KC * DH], f32r, tag="wv")
            tabA = wp.tile([128, T], f32, tag="tabA")
            tabB = wp.tile([128, T], f32, tag="tabB")
            nc.sync.dma_start(tabA[:], ropeA)
            nc.sync.dma_start(tabB[:], ropeB)

            SLAB = 4
            NSLAB = KC // SLAB

            def load_w_slab(s):
                for k in range(s * SLAB, (s + 1) * SLAB):
                    nc.sync.dma_start(wq_sb[:, k * FPC:(k + 1) * FPC],
                                      wq[k * 128:(k + 1) * 128, :])
                    nc.sync.dma_start(wk_sb[:, k * DH:(k + 1) * DH],
                                      wk[k * 128:(k + 1) * 128, :])
                    nc.sync.dma_start(wv_sb[:, k * DH:(k + 1) * DH],
                                      wv[k * 128:(k + 1) * 128, :])

            xsp = sa.enter_context(tc.tile_pool(name="xs", bufs=2))
            evp = sa.enter_context(tc.tile_pool(name="ev", bufs=1))
            psA = sa.enter_context(tc.tile_pool(name="psA", bufs=1, space="PSUM"))

            def stationary(m, k):
                if m < HPC:
                    return wq_sb[:, k * FPC + m * 128: k * FPC + (m + 1) * 128]
                if m == HPC:
                    return wk_sb[:, k * DH:(k + 1) * DH]
                return wv_sb[:, k * DH:(k + 1) * DH]

            pending_vt = None

            def flush_vt():
                nonlocal pending_vt
                if pending_vt is None:
                    return
                vt_p, n_p = pending_vt
                b_p = n_p // 4
                for i in range(4):
                    ptr = psA.tile([128, 128], f32r, tag="tr", bufs=2, name="ptr")
                    nc.tensor.transpose(ptr[:], vt_p[:, i * 128:(i + 1) * 128], ident[:])
                    c_local = 4 * (n_p % 4) + i
                    nc.scalar.copy(Vb[b_p][:, c_local * 128:c_local * 128 + 128], ptr[:])
                pending_vt = None

            for n in range(NT // 512):
                b, tloc = n // 4, 512 * (n % 4)
                ps = [psA.tile([128, 512], f32, tag=f"ps{m}", name=f"ps{m}")
                      for m in range(6)]
                for s in range(NSLAB):
                    if n == 0:
                        load_w_slab(s)
                    xsl = xsp.tile([128, SLAB * 512], f32r, tag="xs", name="xsl")
                    for j in range(SLAB):
                        k = s * SLAB + j
                        nc.sync.dma_start(xsl[:, j * 512:(j + 1) * 512],
                                          xT[k * 128:(k + 1) * 128, n * 512:(n + 1) * 512])
                    for m in range(6):
                        for j in range(SLAB):
                            k = s * SLAB + j
                            nc.tensor.matmul(ps[m][:], stationary(m, k),
                                             xsl[:, j * 512:(j + 1) * 512],
                                             start=(k == 0), stop=(k == KC - 1))
                    if s == 0:
                        flush_vt()   # prev n-tile's V transposes, PE already warm here
                # evict: ACT copies first so PSUM banks free at ACT pace
                qes = []
                for m in range(5):
                    qe = evp.tile([128, 512], f32, tag="qe", bufs=6, name=f"qe{m}")
                    nc.scalar.copy(qe[:], ps[m][:])
                    qes.append(qe)
                vt = evp.tile([128, 512], f32r, tag="vt", bufs=2, name="vt")
                nc.scalar.copy(vt[:], ps[5][:])
                pending_vt = (vt, n)
                # rope chains on DVE
                tA = tabA[:, tloc:tloc + 512]
                tB = tabB[:, tloc:tloc + 512]
                for m in range(5):
                    qe = qes[m]
                    sw = evp.tile([128, 512], f32, tag="sw", bufs=1, name="sw")
                    nc.vector.tensor_copy(sw[0:64, :], qe[64:128, :])
                    nc.vector.tensor_copy(sw[64:128, :], qe[0:64, :])
                    mm = evp.tile([128, 512], f32, tag="mm", bufs=1, name="mm")
                    nc.vector.tensor_mul(mm[:], sw[:], tB)
                    tt = evp.tile([128, 512], f32, tag="tt", bufs=1, name="tt")
                    nc.vector.tensor_mul(tt[:], qe[:], tA)
                    if m < HPC:
                        ro = evp.tile([128, 512], f32r, tag="ro", bufs=2, name="ro")
                        nc.vector.tensor_add(ro[:], tt[:], mm[:])
                        nc.sync.dma_start(qTd[m * 128:(m + 1) * 128, n * 512:(n + 1) * 512], ro[:])
                    else:
                        nc.vector.tensor_add(KTb[b][:, tloc:tloc + 512], tt[:], mm[:])
            flush_vt()

        # ---------------- Phase B: attention -----------------
        with ExitStack() as sbc:
            wop = sbc.enter_context(tc.tile_pool(name="wop", bufs=1))
            wo_sb = wop.tile([128, HPC * D], f32r, tag="wo")
            msk = wop.tile([128, 4 * 512], f32r, tag="msk")
            for r in range(4):
                nc.sync.dma_start(msk[:, r * 512:(r + 1) * 512], masks[r * 128:(r + 1) * 128, :])

            sb = sbc.enter_context(ExitStack())
            qtp = sb.enter_context(tc.tile_pool(name="qtp", bufs=3))
            sxp = sb.enter_context(tc.tile_pool(name="sxp", bufs=6))
            smp = sb.enter_context(tc.tile_pool(name="smp", bufs=2))
            psB = sb.enter_context(tc.tile_pool(name="psB", bufs=2, space="PSUM"))

            pending_norm = None

            def flush_norm():
                nonlocal pending_norm
                if pending_norm is None:
                    return
                ps_ctx_p, ps_sm_p, h_p, b_p, qt_p = pending_norm
                rs = smp.tile([1, 512], f32r, tag="rs", name="rs")
                with nc.allow_low_precision(reason="f32r recip for broadcast mm"):
                    nc.vector.reciprocal(rs[:], ps_sm_p[:])
                # broadcast rs across partitions: ones_row^T @ rs (K=1 matmul)
                ps_bc = psB.tile([128, 512], f32, tag="bc", bufs=1, name="ps_bc")
                nc.tensor.matmul(ps_bc[:], ones_row[:], rs[:], start=True, stop=True)
                bcs = smp.tile([128, 512], f32, tag="bcs", name="bcs")
                nc.scalar.copy(bcs[:], ps_bc[:])
                cb = smp.tile([128, 512], f32r, tag="cb", name="cb")
                nc.vector.tensor_mul(cb[:], ps_ctx_p[:], bcs[:])
                nc.sync.dma_start(
                    ctxd[h_p * 128:(h_p + 1) * 128,
                         b_p * T + qt_p * 512: b_p * T + (qt_p + 1) * 512],
                    cb[:])
                pending_norm = None

            for b in range(B):
                for h in range(HPC):
                    qt_t = qtp.tile([128, T], f32r, tag="qt")
                    nc.sync.dma_start(qt_t[:], qTd[h * 128:(h + 1) * 128, b * T:(b + 1) * T])
                    for qt in range(4):
                        ps_ctx = psB.tile([128, 512], f32, tag="ctx")
                        ps_sm = psB.tile([1, 512], f32, tag="sm")
                        nk = 4 * qt + 4

                        def issue_st(kt):
                            ps_st = psB.tile([128, 512], f32, tag="st", bufs=3, name="ps_st")
                            nc.tensor.matmul(ps_st[:],
                                             KTb[b][:, kt * 128:(kt + 1) * 128],
                                             qt_t[:, qt * 512:(qt + 1) * 512],
                                             start=True, stop=True)
                            se = sxp.tile([128, 512], f32r, tag="se", name="se")
                            nc.scalar.activation(se[:], ps_st[:],
                                                 mybir.ActivationFunctionType.Exp,
                                                 scale=EXP_SCALE)
                            if kt >= 4 * qt:
                                r = kt - 4 * qt
                                nc.vector.tensor_mul(se[:], se[:], msk[:, r * 512:(r + 1) * 512])
                            return se

                        se_q = [issue_st(0), issue_st(1)]
                        for kt in range(nk):
                            se_cur = se_q.pop(0)
                            if kt + 2 < nk:
                                se_q.append(issue_st(kt + 2))
                            nc.tensor.matmul(ps_ctx[:],
                                             Vb[b][:, kt * 128:(kt + 1) * 128],
                                             se_cur[:], start=(kt == 0), stop=(kt == nk - 1))
                            nc.tensor.matmul(ps_sm[:], ones[:], se_cur[:],
                                             start=(kt == 0), stop=(kt == nk - 1))
                            if kt == 2:
                                flush_norm()  # prev q-tile's normalize, PE already busy
                        pending_norm = (ps_ctx, ps_sm, h, b, qt)
            flush_norm()

            sb.close()

            for h in range(HPC):
                nc.sync.dma_start(wo_sb[:, h * D:(h + 1) * D], wo[h * 128:(h + 1) * 128, :])

            # ---------------- Phase C: output projection -----------------
            with ExitStack() as sc:
                cmp_ = sc.enter_context(tc.tile_pool(name="cmp", bufs=5))
                obp = sc.enter_context(tc.tile_pool(name="obp", bufs=4))
                psC = sc.enter_context(tc.tile_pool(name="psC", bufs=4, space="PSUM"))
                for m in range(NT // 128):
                    cm = cmp_.tile([128, FPC], f32r, tag="cm")
                    for h in range(HPC):
                        nc.sync.dma_start(cm[:, h * 128:(h + 1) * 128],
                                          ctxd[h * 128:(h + 1) * 128, m * 128:(m + 1) * 128])
                    for n in range(D // 512):
                        pso = psC.tile([128, 512], f32, tag="oc")
                        for h in range(HPC):
                            nc.tensor.matmul(pso[:], cm[:, h * 128:(h + 1) * 128],
                                             wo_sb[:, h * D + n * 512: h * D + (n + 1) * 512],
                                             start=(h == 0), stop=(h == HPC - 1))
                        ob = obp.tile([128, 512], f32, tag="ob")
                        nc.scalar.copy(ob[:], pso[:])
                        nc.sync.dma_start(outp[m * 128:(m + 1) * 128, n * 512:(n + 1) * 512], ob[:])

    nc.compile()
    return nc


def _get_nc():
    if "nc" not in _NC_CACHE:
        _NC_CACHE["nc"] = _build_program()
    return _NC_CACHE["nc"]


def _rope_tables():
    j = np.arange(0, DH, 2, dtype=np.float32) / np.float32(DH)
    inv_freq = (np.float32(1.0) / (np.float32(ROPE_BASE) ** j)).astype(np.float32)
    t = np.arange(T, dtype=np.float32)
    freqs = np.outer(t, inv_freq).astype(np.float32)   # (T, 64)
    c = np.cos(freqs).astype(np.float32).T             # (64, T)
    s = np.sin(freqs).astype(np.float32).T
    A = np.vstack([c, c]).astype(np.float32)           # (128, T)
    Bt = np.vstack([-s, s]).astype(np.float32)
    return np.ascontiguousarray(A), np.ascontiguousarray(Bt)


def _causal_masks():
    m = np.zeros((512, 512), dtype=np.float32)
    for r in range(4):
        p = np.arange(128)[:, None]
        f = np.arange(512)[None, :]
        m[r * 128:(r + 1) * 128, :] = (r * 128 + p <= f).astype(np.float32)
    return m


def kernel(x, Wq, Wk, Wv, Wo):
    x = np.asarray(x, dtype=np.float32)
    Wq = np.asarray(Wq, dtype=np.float32)
    Wk = np.asarray(Wk, dtype=np.float32)
    Wv = np.asarray(Wv, dtype=np.float32)
    Wo = np.asarray(Wo, dtype=np.float32)

    nc = _get_nc()

    xT = _round_fp32r(x.reshape(NT, D).T)
    A, Bt = _rope_tables()
    msk = _round_fp32r(_causal_masks())

    in_maps = []
    for g in range(8):
        in_maps.append({
            "xT": xT,
            "wq": _round_fp32r(Wq[:, g * FPC:(g + 1) * FPC]),
            "wk": _round_fp32r(Wk[:, g * DH:(g + 1) * DH]),
            "wv": _round_fp32r(Wv[:, g * DH:(g + 1) * DH]),
            "wo": _round_fp32r(Wo[g * FPC:(g + 1) * FPC, :]),
            "ropeA": A,
            "ropeB": Bt,
            "masks": msk,
        })

    res = run_bass_kernel_spmd(nc, in_maps, list(range(8)))
    acc = res.results[0]["outp"].astype(np.float32)
    for g in range(1, 8):
        acc = acc + res.results[g]["outp"]
    return np.ascontiguousarray(acc.reshape(B, T, D), dtype=np.float32)



# revision 4
# speedup vs baseline: 1.2372x; 1.2372x over previous
"""GroupedQueryAttention TRN2 Bass kernel, sharded over 8 NeuronCores.

Problem (hardcoded): B=2, T=2048, D=4096, 32 Q heads x 128, 8 KV groups x 128,
RoPE (base 5e5), causal, out = ctx @ Wo.

Sharding: core g owns Q heads 4g..4g+3 (Wq columns 512g:512g+512), KV group g
(Wk/Wv columns 128g:128g+128), and Wo rows 512g:512g+512 (row-parallel).
Each core computes a full-shape partial output; host sums the 8 partials.

v2: all-bf16 data path (f32 PSUM accumulation), fully SBUF-resident
intermediates (Q / K^T / V / ctx never round-trip DRAM), causal narrowing of
diagonal score blocks, fast-reciprocal softmax normalization off the PE
critical path.
"""
import sys
import numpy as np

for _p in ("/opt/trn_rl_repo", "/root/.axon_site", "/root/.axon_site/_ro/trn_rl_repo"):
    if _p not in sys.path:
        sys.path.append(_p)

from contextlib import ExitStack

import ml_dtypes

import concourse.bass as bass
import concourse.tile as tile
from concourse import bacc, mybir
from concourse.bass_utils import run_bass_kernel_spmd
from concourse.masks import make_identity

B, T, D = 2, 2048, 4096
NH, NKV, DH = 32, 8, 128
HPC = NH // 8          # 4 q heads per core
FPC = HPC * DH         # 512 q features per core
ROPE_BASE = 500000.0
NT = B * T             # 4096 tokens
f32 = mybir.dt.float32
bf16 = mybir.dt.bfloat16
EXP_SCALE = 1.0 / float(np.sqrt(DH))
BF = ml_dtypes.bfloat16

_NC_CACHE = {}


def _build_program():
    nc = bacc.Bacc("TRN2", target_bir_lowering=False, debug=False)

    xT = nc.dram_tensor("xT", [D, NT], bf16, kind="ExternalInput").ap()
    wq = nc.dram_tensor("wq", [D, FPC], bf16, kind="ExternalInput").ap()
    wk = nc.dram_tensor("wk", [D, DH], bf16, kind="ExternalInput").ap()
    wv = nc.dram_tensor("wv", [D, DH], bf16, kind="ExternalInput").ap()
    wo = nc.dram_tensor("wo", [FPC, D], bf16, kind="ExternalInput").ap()
    ropeA = nc.dram_tensor("ropeA", [128, T], bf16, kind="ExternalInput").ap()
    ropeB = nc.dram_tensor("ropeB", [128, T], bf16, kind="ExternalInput").ap()
    masks = nc.dram_tensor("masks", [512, 512], bf16, kind="ExternalInput").ap()
    outp = nc.dram_tensor("outp", [NT, D], bf16, kind="ExternalOutput").ap()

    KC = D // 128  # 32 contraction chunks

    with tile.TileContext(nc) as tc, ExitStack() as s0:
        kvp = s0.enter_context(tc.tile_pool(name="kv", bufs=1))
        # persistent SBUF-resident intermediates
        Qh = [kvp.tile([128, NT], bf16, tag=f"Q{h}", name=f"Q{h}") for h in range(HPC)]
        KTb = [kvp.tile([128, T], bf16, tag=f"KT{i}", name=f"KT{i}") for i in range(B)]
        Vb = [kvp.tile([128, T], bf16, tag=f"V{i}", name=f"V{i}") for i in range(B)]
        Ch = [kvp.tile([128, NT], bf16, tag=f"C{h}", name=f"C{h}") for h in range(HPC)]
        tabA = kvp.tile([128, T], bf16, tag="tabA")
        tabB = kvp.tile([128, T], bf16, tag="tabB")
        msk = kvp.tile([128, 4 * 512], bf16, tag="msk")
        nc.sync.dma_start(tabA[:], ropeA)
        nc.sync.dma_start(tabB[:], ropeB)
        for r in range(4):
            nc.sync.dma_start(msk[:, r * 512:(r + 1) * 512], masks[r * 128:(r + 1) * 128, :])

        ident_f = kvp.tile([128, 128], f32, tag="ident_f")
        make_identity(nc, ident_f[:])
        ident = kvp.tile([128, 128], bf16, tag="ident")
        nc.vector.tensor_copy(ident[:], ident_f[:])
        ones_f = kvp.tile([128, 1], f32, tag="ones_f")
        nc.vector.memset(ones_f[:], 1.0)
        ones = kvp.tile([128, 1], bf16, tag="ones")
        nc.vector.tensor_copy(ones[:], ones_f[:])
        ones_row_f = kvp.tile([1, 128], f32, tag="ones_row_f")
        nc.vector.memset(ones_row_f[:], 1.0)
        ones_row = kvp.tile([1, 128], bf16, tag="ones_row")
        nc.vector.tensor_copy(ones_row[:], ones_row_f[:])

        # ---------------- Phase A: projections + RoPE -----------------
        with ExitStack() as sa:
            wp = sa.enter_context(tc.tile_pool(name="wts", bufs=1))
            wq_sb = wp.tile([128, KC * FPC], bf16, tag="wq")
            wk_sb = wp.tile([128, KC * DH], bf16, tag="wk")
            wv_sb = wp.tile([128, KC * DH], bf16, tag="wv")

            SLAB = 4
            NSLAB = KC // SLAB

            def load_w_slab(s):
                for k in range(s * SLAB, (s + 1) * SLAB):
                    nc.sync.dma_start(wq_sb[:, k * FPC:(k + 1) * FPC],
                                      wq[k * 128:(k + 1) * 128, :])
                    nc.sync.dma_start(wk_sb[:, k * DH:(k + 1) * DH],
                                      wk[k * 128:(k + 1) * 128, :])
                    nc.sync.dma_start(wv_sb[:, k * DH:(k + 1) * DH],
                                      wv[k * 128:(k + 1) * 128, :])

            xsp = sa.enter_context(tc.tile_pool(name="xs", bufs=2))
            evp = sa.enter_context(tc.tile_pool(name="ev", bufs=1))
            psA = sa.enter_context(tc.tile_pool(name="psA", bufs=1, space="PSUM"))

            def stationary(m, k):
                if m < HPC:
                    return wq_sb[:, k * FPC + m * 128: k * FPC + (m + 1) * 128]
                if m == HPC:
                    return wk_sb[:, k * DH:(k + 1) * DH]
                return wv_sb[:, k * DH:(k + 1) * DH]

            pending_vt = None

            def flush_vt():
                nonlocal pending_vt
                if pending_vt is None:
                    return
                vt_p, n_p = pending_vt
                b_p = n_p // 4
                for i in range(4):
                    ptr = psA.tile([128, 128], bf16, tag="tr", bufs=2, name="ptr")
                    nc.tensor.transpose(ptr[:], vt_p[:, i * 128:(i + 1) * 128], ident[:])
                    c_local = 4 * (n_p % 4) + i
                    nc.scalar.copy(Vb[b_p][:, c_local * 128:c_local * 128 + 128], ptr[:])
                pending_vt = None

            for n in range(NT // 512):
                b, tloc = n // 4, 512 * (n % 4)
                ps = [psA.tile([128, 512], f32, tag=f"ps{m}", name=f"ps{m}")
                      for m in range(6)]
                for s in range(NSLAB):
                    if n == 0:
                        load_w_slab(s)
                    xsl = xsp.tile([128, SLAB * 512], bf16, tag="xs", name="xsl")
                    for j in range(SLAB):
                        k = s * SLAB + j
                        nc.sync.dma_start(xsl[:, j * 512:(j + 1) * 512],
                                          xT[k * 128:(k + 1) * 128, n * 512:(n + 1) * 512])
                    for m in range(6):
                        for j in range(SLAB):
                            k = s * SLAB + j
                            nc.tensor.matmul(ps[m][:], stationary(m, k),
                                             xsl[:, j * 512:(j + 1) * 512],
                                             start=(k == 0), stop=(k == KC - 1))
                    if s == 0:
                        flush_vt()   # prev n-tile's V transposes, PE already warm here
                # evict: ACT copies first so PSUM banks free at ACT pace
                qes = []
                for m in range(5):
                    qe = evp.tile([128, 512], bf16, tag="qe", bufs=6, name=f"qe{m}")
                    nc.scalar.copy(qe[:], ps[m][:])
                    qes.append(qe)
                vt = evp.tile([128, 512], bf16, tag="vt", bufs=2, name="vt")
                nc.scalar.copy(vt[:], ps[5][:])
                pending_vt = (vt, n)
                # rope chains on DVE (bf16, 2x rate)
                tA = tabA[:, tloc:tloc + 512]
                tB = tabB[:, tloc:tloc + 512]
                for m in range(5):
                    qe = qes[m]
                    sw = evp.tile([128, 512], bf16, tag="sw", bufs=1, name="sw")
                    nc.vector.tensor_copy(sw[0:64, :], qe[64:128, :])
                    nc.vector.tensor_copy(sw[64:128, :], qe[0:64, :])
                    mm = evp.tile([128, 512], bf16, tag="mm", bufs=1, name="mm")
                    nc.vector.tensor_mul(mm[:], sw[:], tB)
                    tt = evp.tile([128, 512], bf16, tag="tt", bufs=1, name="tt")
                    nc.vector.tensor_mul(tt[:], qe[:], tA)
                    if m < HPC:
                        nc.vector.tensor_add(Qh[m][:, n * 512:(n + 1) * 512], tt[:], mm[:])
                    else:
                        nc.vector.tensor_add(KTb[b][:, tloc:tloc + 512], tt[:], mm[:])
            flush_vt()

        # ---------------- Phase B: attention -----------------
        with ExitStack() as sbc:
            wop = sbc.enter_context(tc.tile_pool(name="wop", bufs=1))
            wo_sb = wop.tile([128, HPC * D], bf16, tag="wo")
            for h in range(HPC):
                nc.sync.dma_start(wo_sb[:, h * D:(h + 1) * D], wo[h * 128:(h + 1) * 128, :])

            sb = sbc.enter_context(ExitStack())
            sxp = sb.enter_context(tc.tile_pool(name="sxp", bufs=6))
            smp = sb.enter_context(tc.tile_pool(name="smp", bufs=2))
            psB = sb.enter_context(tc.tile_pool(name="psB", bufs=2, space="PSUM"))

            pending_norm = None
            cols_since_flush = 0

            def flush_norm():
                nonlocal pending_norm
                if pending_norm is None:
                    return
                ps_ctx_p, ps_sm_p, h_p, b_p, qt_p = pending_norm
                rs = smp.tile([1, 512], f32, tag="rs", name="rs")
                nc.vector.reciprocal_approx_fast(out=rs[:], in_=ps_sm_p[:])
                rs_b = smp.tile([1, 512], bf16, tag="rsb", name="rsb")
                nc.scalar.copy(rs_b[:], rs[:])
                # broadcast recip across partitions: ones_row^T @ rs_b (K=1 matmul)
                ps_bc = psB.tile([128, 512], f32, tag="bc", bufs=1, name="ps_bc")
                nc.tensor.matmul(ps_bc[:], ones_row[:], rs_b[:], start=True, stop=True)
                bcs = smp.tile([128, 512], bf16, tag="bcs", name="bcs")
                nc.scalar.copy(bcs[:], ps_bc[:])
                nc.vector.tensor_mul(
                    Ch[h_p][:, b_p * T + qt_p * 512: b_p * T + (qt_p + 1) * 512],
                    ps_ctx_p[:], bcs[:])
                pending_norm = None

            for b in range(B):
                for h in range(HPC):
                    qt_full = Qh[h][:, b * T:(b + 1) * T]
                    for qt in range(4):
                        ps_ctx = psB.tile([128, 512], f32, tag="ctx")
                        ps_sm = psB.tile([1, 512], f32, tag="sm")
                        # kt plan: full-width blocks then narrowed diagonals
                        # (kt, q-offset within the 512 q-window, width)
                        plan = [(kt, 0, 512) for kt in range(4 * qt)]
                        plan += [(4 * qt + r, 128 * r if r else 0, 512 - 128 * r if r else 512)
                                 for r in range(4)]

                        def issue_st(idx):
                            kt, off, w = plan[idx]
                            ps_st = psB.tile([128, 512], f32, tag="st", bufs=3, name="ps_st")
                            nc.tensor.matmul(ps_st[:, 0:w],
                                             KTb[b][:, kt * 128:(kt + 1) * 128],
                                             qt_full[:, qt * 512 + off: qt * 512 + off + w],
                                             start=True, stop=True)
                            se = sxp.tile([128, 512], bf16, tag="se", name="se")
                            nc.scalar.activation(se[:, 0:w], ps_st[:, 0:w],
                                                 mybir.ActivationFunctionType.Exp,
                                                 scale=EXP_SCALE)
                            r = kt - 4 * qt
                            if r >= 0:
                                nc.vector.tensor_mul(
                                    se[:, 0:w], se[:, 0:w],
                                    msk[:, r * 512 + off: r * 512 + 512])
                            return se

                        n_plan = len(plan)
                        se_q = [issue_st(0), issue_st(1)]
                        for idx in range(n_plan):
                            kt, off, w = plan[idx]
                            se_cur = se_q.pop(0)
                            if idx + 2 < n_plan:
                                se_q.append(issue_st(idx + 2))
                            last = (idx == n_plan - 1)
                            nc.tensor.matmul(ps_ctx[:, off:off + w],
                                             Vb[b][:, kt * 128:(kt + 1) * 128],
                                             se_cur[:, 0:w],
                                             start=(idx == 0), stop=last,
                                             skip_group_check=True)
                            nc.tensor.matmul(ps_sm[:, off:off + w], ones[:],
                                             se_cur[:, 0:w],
                                             start=(idx == 0), stop=last,
                                             skip_group_check=True)
                            cols_since_flush += 3 * w
                            if pending_norm is not None and cols_since_flush >= 5000:
                                flush_norm()
                        flush_norm()  # lead time ran out: flush before re-pending
                        pending_norm = (ps_ctx, ps_sm, h, b, qt)
                        cols_since_flush = 0
            flush_norm()
            sb.close()

            # ---------------- Phase C: output projection -----------------
            with ExitStack() as sc:
                obp = sc.enter_context(tc.tile_pool(name="obp", bufs=4))
                psC = sc.enter_context(tc.tile_pool(name="psC", bufs=4, space="PSUM"))
                for m in range(NT // 128):
                    for n in range(D // 512):
                        pso = psC.tile([128, 512], f32, tag="oc")
                        for h in range(HPC):
                            nc.tensor.matmul(pso[:], Ch[h][:, m * 128:(m + 1) * 128],
                                             wo_sb[:, h * D + n * 512: h * D + (n + 1) * 512],
                                             start=(h == 0), stop=(h == HPC - 1))
                        ob = obp.tile([128, 512], bf16, tag="ob")
                        nc.scalar.copy(ob[:], pso[:])
                        nc.sync.dma_start(outp[m * 128:(m + 1) * 128, n * 512:(n + 1) * 512], ob[:])

    nc.compile()
    return nc


def _get_nc():
    if "nc" not in _NC_CACHE:
        _NC_CACHE["nc"] = _build_program()
    return _NC_CACHE["nc"]


def _rope_tables():
    j = np.arange(0, DH, 2, dtype=np.float32) / np.float32(DH)
    inv_freq = (np.float32(1.0) / (np.float32(ROPE_BASE) ** j)).astype(np.float32)
    t = np.arange(T, dtype=np.float32)
    freqs = np.outer(t, inv_freq).astype(np.float32)   # (T, 64)
    c = np.cos(freqs).astype(np.float32).T             # (64, T)
    s = np.sin(freqs).astype(np.float32).T
    A = np.vstack([c, c]).astype(np.float32)           # (128, T)
    Bt = np.vstack([-s, s]).astype(np.float32)
    return np.ascontiguousarray(A).astype(BF), np.ascontiguousarray(Bt).astype(BF)


def _causal_masks():
    m = np.zeros((512, 512), dtype=np.float32)
    for r in range(4):
        p = np.arange(128)[:, None]
        f = np.arange(512)[None, :]
        m[r * 128:(r + 1) * 128, :] = (r * 128 + p <= f).astype(np.float32)
    return m.astype(BF)


def _make_in_maps(x, Wq, Wk, Wv, Wo):
    xT = np.ascontiguousarray(x.reshape(NT, D).T).astype(BF)
    A, Bt = _rope_tables()
    msk = _causal_masks()

    in_maps = []
    for g in range(8):
        in_maps.append({
            "xT": xT,
            "wq": np.ascontiguousarray(Wq[:, g * FPC:(g + 1) * FPC]).astype(BF),
            "wk": np.ascontiguousarray(Wk[:, g * DH:(g + 1) * DH]).astype(BF),
            "wv": np.ascontiguousarray(Wv[:, g * DH:(g + 1) * DH]).astype(BF),
            "wo": np.ascontiguousarray(Wo[g * FPC:(g + 1) * FPC, :]).astype(BF),
            "ropeA": A,
            "ropeB": Bt,
            "masks": msk,
        })
    return in_maps


def kernel(x, Wq, Wk, Wv, Wo):
    x = np.asarray(x, dtype=np.float32)
    Wq = np.asarray(Wq, dtype=np.float32)
    Wk = np.asarray(Wk, dtype=np.float32)
    Wv = np.asarray(Wv, dtype=np.float32)
    Wo = np.asarray(Wo, dtype=np.float32)

    nc = _get_nc()
    in_maps = _make_in_maps(x, Wq, Wk, Wv, Wo)

    res = run_bass_kernel_spmd(nc, in_maps, list(range(8)))
    acc = res.results[0]["outp"].astype(np.float32)
    for g in range(1, 8):
        acc = acc + res.results[g]["outp"].astype(np.float32)
    return np.ascontiguousarray(acc.reshape(B, T, D), dtype=np.float32)


# revision 5
# speedup vs baseline: 1.3013x; 1.0518x over previous
"""GroupedQueryAttention TRN2 Bass kernel, sharded over 8 NeuronCores.

Problem (hardcoded): B=2, T=2048, D=4096, 32 Q heads x 128, 8 KV groups x 128,
RoPE (base 5e5), causal, out = ctx @ Wo.

Sharding: core g owns Q heads 4g..4g+3 (Wq columns 512g:512g+512), KV group g
(Wk/Wv columns 128g:128g+128), and Wo rows 512g:512g+512 (row-parallel).
Each core computes a full-shape partial output; host sums the 8 partials.

v3: all-bf16 data path (f32 PSUM accumulation), fully SBUF-resident
intermediates, causal narrowing of diagonal score blocks, additive causal mask
applied in PSUM via an identity matmul (keeps the score->exp->ctx chain off
the DVE), depth-3 exp prefetch, consolidated slab DMAs, fast-reciprocal
softmax normalization off the PE critical path.
"""
import sys
import numpy as np

for _p in ("/opt/trn_rl_repo", "/root/.axon_site", "/root/.axon_site/_ro/trn_rl_repo"):
    if _p not in sys.path:
        sys.path.append(_p)

from contextlib import ExitStack

import ml_dtypes

import concourse.bass as bass
import concourse.tile as tile
from concourse import bacc, mybir
from concourse.bass_utils import run_bass_kernel_spmd
from concourse.masks import make_identity

B, T, D = 2, 2048, 4096
NH, NKV, DH = 32, 8, 128
HPC = NH // 8          # 4 q heads per core
FPC = HPC * DH         # 512 q features per core
ROPE_BASE = 500000.0
NT = B * T             # 4096 tokens
f32 = mybir.dt.float32
bf16 = mybir.dt.bfloat16
EXP_SCALE = 1.0 / float(np.sqrt(DH))
MASK_NEG = -20000.0
BF = ml_dtypes.bfloat16

_NC_CACHE = {}


def _build_program():
    nc = bacc.Bacc("TRN2", target_bir_lowering=False, debug=False)

    xT = nc.dram_tensor("xT", [D, NT], bf16, kind="ExternalInput").ap()
    wq = nc.dram_tensor("wq", [D, FPC], bf16, kind="ExternalInput").ap()
    wk = nc.dram_tensor("wk", [D, DH], bf16, kind="ExternalInput").ap()
    wv = nc.dram_tensor("wv", [D, DH], bf16, kind="ExternalInput").ap()
    wo = nc.dram_tensor("wo", [FPC, D], bf16, kind="ExternalInput").ap()
    ropeA = nc.dram_tensor("ropeA", [128, T], bf16, kind="ExternalInput").ap()
    ropeB = nc.dram_tensor("ropeB", [128, T], bf16, kind="ExternalInput").ap()
    masks = nc.dram_tensor("masks", [512, 512], bf16, kind="ExternalInput").ap()
    outp = nc.dram_tensor("outp", [NT, D], bf16, kind="ExternalOutput").ap()

    KC = D // 128  # 32 contraction chunks
    SLAB = 4
    NSLAB = KC // SLAB

    with tile.TileContext(nc) as tc, ExitStack() as s0:
        kvp = s0.enter_context(tc.tile_pool(name="kv", bufs=1))
        # persistent SBUF-resident intermediates
        Qh = [kvp.tile([128, NT], bf16, tag=f"Q{h}", name=f"Q{h}") for h in range(HPC)]
        KTb = [kvp.tile([128, T], bf16, tag=f"KT{i}", name=f"KT{i}") for i in range(B)]
        Vb = [kvp.tile([128, T], bf16, tag=f"V{i}", name=f"V{i}") for i in range(B)]
        Ch = [kvp.tile([128, NT], bf16, tag=f"C{h}", name=f"C{h}") for h in range(HPC)]
        tabA = kvp.tile([128, T], bf16, tag="tabA")
        tabB = kvp.tile([128, T], bf16, tag="tabB")
        amsk = kvp.tile([128, 4 * 512], bf16, tag="amsk")

        with ExitStack() as sa:
            wp = sa.enter_context(tc.tile_pool(name="wts", bufs=1))
            wq_sb = wp.tile([128, KC * FPC], bf16, tag="wq")
            wk_sb = wp.tile([128, KC * DH], bf16, tag="wk")
            wv_sb = wp.tile([128, KC * DH], bf16, tag="wv")
            xsp = sa.enter_context(tc.tile_pool(name="xs", bufs=3))
            evp = sa.enter_context(tc.tile_pool(name="ev", bufs=1))
            psA = sa.enter_context(tc.tile_pool(name="psA", bufs=1, space="PSUM"))

            def load_w_slab(s):
                # one consolidated DMA per weight tensor per 4-chunk slab
                nc.sync.dma_start(
                    wq_sb[:, s * SLAB * FPC:(s + 1) * SLAB * FPC]
                    .rearrange("p (k c) -> p k c", k=SLAB),
                    wq[s * 512:(s + 1) * 512, :]
                    .rearrange("(k p) c -> p k c", k=SLAB))
                nc.sync.dma_start(
                    wk_sb[:, s * SLAB * DH:(s + 1) * SLAB * DH]
                    .rearrange("p (k c) -> p k c", k=SLAB),
                    wk[s * 512:(s + 1) * 512, :]
                    .rearrange("(k p) c -> p k c", k=SLAB))
                nc.sync.dma_start(
                    wv_sb[:, s * SLAB * DH:(s + 1) * SLAB * DH]
                    .rearrange("p (k c) -> p k c", k=SLAB),
                    wv[s * 512:(s + 1) * 512, :]
                    .rearrange("(k p) c -> p k c", k=SLAB))

            def load_xsl(n, s):
                xsl = xsp.tile([128, SLAB * 512], bf16, tag="xs", name="xsl")
                nc.sync.dma_start(
                    xsl[:].rearrange("p (k c) -> p k c", k=SLAB),
                    xT[s * 512:(s + 1) * 512, n * 512:(n + 1) * 512]
                    .rearrange("(k p) c -> p k c", k=SLAB))
                return xsl

            # critical-path first: slab 0 weights + first x slab + slab 1
            load_w_slab(0)
            xsl0 = load_xsl(0, 0)
            load_w_slab(1)

            # setup that doesn't gate the first matmuls
            ident_f = kvp.tile([128, 128], f32, tag="ident_f")
            make_identity(nc, ident_f[:])
            ident = kvp.tile([128, 128], bf16, tag="ident")
            nc.vector.tensor_copy(ident[:], ident_f[:])
            ones_f = kvp.tile([128, 1], f32, tag="ones_f")
            nc.vector.memset(ones_f[:], 1.0)
            ones = kvp.tile([128, 1], bf16, tag="ones")
            nc.vector.tensor_copy(ones[:], ones_f[:])
            ones_row_f = kvp.tile([1, 128], f32, tag="ones_row_f")
            nc.vector.memset(ones_row_f[:], 1.0)
            ones_row = kvp.tile([1, 128], bf16, tag="ones_row")
            nc.vector.tensor_copy(ones_row[:], ones_row_f[:])

            # ---------------- Phase A: projections + RoPE -----------------
            def stationary(m, k):
                if m < HPC:
                    return wq_sb[:, k * FPC + m * 128: k * FPC + (m + 1) * 128]
                if m == HPC:
                    return wk_sb[:, k * DH:(k + 1) * DH]
                return wv_sb[:, k * DH:(k + 1) * DH]

            pending_vt = None

            def flush_vt():
                nonlocal pending_vt
                if pending_vt is None:
                    return
                vt_p, n_p = pending_vt
                b_p = n_p // 4
                for i in range(4):
                    ptr = psA.tile([128, 128], bf16, tag="tr", bufs=2, name="ptr")
                    nc.tensor.transpose(ptr[:], vt_p[:, i * 128:(i + 1) * 128], ident[:])
                    c_local = 4 * (n_p % 4) + i
                    nc.scalar.copy(Vb[b_p][:, c_local * 128:c_local * 128 + 128], ptr[:])
                pending_vt = None

            for n in range(NT // 512):
                b, tloc = n // 4, 512 * (n % 4)
                ps = [psA.tile([128, 512], f32, tag=f"ps{m}", name=f"ps{m}")
                      for m in range(6)]
                for s in range(NSLAB):
                    xsl = xsl0 if (n == 0 and s == 0) else load_xsl(n, s)
                    if n == 0 and s + 2 < NSLAB:
                        load_w_slab(s + 2)
                    for m in range(6):
                        for j in range(SLAB):
                            k = s * SLAB + j
                            nc.tensor.matmul(ps[m][:], stationary(m, k),
                                             xsl[:, j * 512:(j + 1) * 512],
                                             start=(k == 0), stop=(k == KC - 1))
                    if s == 0:
                        flush_vt()   # prev n-tile's V transposes, PE already warm here
                if n == 0:
                    nc.sync.dma_start(tabA[:], ropeA)
                    nc.sync.dma_start(tabB[:], ropeB)
                # evict: ACT copies first so PSUM banks free at ACT pace
                qes = []
                for m in range(5):
                    qe = evp.tile([128, 512], bf16, tag="qe", bufs=6, name=f"qe{m}")
                    nc.scalar.copy(qe[:], ps[m][:])
                    qes.append(qe)
                vt = evp.tile([128, 512], bf16, tag="vt", bufs=2, name="vt")
                nc.scalar.copy(vt[:], ps[5][:])
                pending_vt = (vt, n)
                # rope chains on DVE (bf16, 2x rate)
                tA = tabA[:, tloc:tloc + 512]
                tB = tabB[:, tloc:tloc + 512]
                for m in range(5):
                    qe = qes[m]
                    sw = evp.tile([128, 512], bf16, tag="sw", bufs=1, name="sw")
                    nc.vector.tensor_copy(sw[0:64, :], qe[64:128, :])
                    nc.vector.tensor_copy(sw[64:128, :], qe[0:64, :])
                    mm = evp.tile([128, 512], bf16, tag="mm", bufs=1, name="mm")
                    nc.vector.tensor_mul(mm[:], sw[:], tB)
                    tt = evp.tile([128, 512], bf16, tag="tt", bufs=1, name="tt")
                    nc.vector.tensor_mul(tt[:], qe[:], tA)
                    if m < HPC:
                        nc.vector.tensor_add(Qh[m][:, n * 512:(n + 1) * 512], tt[:], mm[:])
                    else:
                        nc.vector.tensor_add(KTb[b][:, tloc:tloc + 512], tt[:], mm[:])
            flush_vt()

        # ---------------- Phase B: attention -----------------
        with ExitStack() as sbc:
            wop = sbc.enter_context(tc.tile_pool(name="wop", bufs=1))
            wo_sb = wop.tile([128, HPC * D], bf16, tag="wo")
            for r in range(4):
                nc.sync.dma_start(amsk[:, r * 512:(r + 1) * 512],
                                  masks[r * 128:(r + 1) * 128, :])
            for h in range(HPC):
                nc.sync.dma_start(wo_sb[:, h * D:(h + 1) * D], wo[h * 128:(h + 1) * 128, :])

            sb = sbc.enter_context(ExitStack())
            sxp = sb.enter_context(tc.tile_pool(name="sxp", bufs=8))
            smp = sb.enter_context(tc.tile_pool(name="smp", bufs=2))
            psB = sb.enter_context(tc.tile_pool(name="psB", bufs=2, space="PSUM"))

            pending_norm = None
            cols_since_flush = 0

            def flush_norm():
                nonlocal pending_norm
                if pending_norm is None:
                    return
                ps_ctx_p, ps_sm_p, h_p, b_p, qt_p = pending_norm
                rs = smp.tile([1, 512], f32, tag="rs", name="rs")
                nc.vector.reciprocal_approx_fast(out=rs[:], in_=ps_sm_p[:])
                rs_b = smp.tile([1, 512], bf16, tag="rsb", name="rsb")
                nc.scalar.copy(rs_b[:], rs[:])
                # broadcast recip across partitions: ones_row^T @ rs_b (K=1 matmul)
                ps_bc = psB.tile([128, 512], f32, tag="st", bufs=4, name="ps_bc")
                nc.tensor.matmul(ps_bc[:], ones_row[:], rs_b[:], start=True, stop=True)
                bcs = smp.tile([128, 512], bf16, tag="bcs", name="bcs")
                nc.scalar.copy(bcs[:], ps_bc[:])
                nc.vector.tensor_mul(
                    Ch[h_p][:, b_p * T + qt_p * 512: b_p * T + (qt_p + 1) * 512],
                    ps_ctx_p[:], bcs[:])
                pending_norm = None

            for b in range(B):
                for h in range(HPC):
                    qt_full = Qh[h][:, b * T:(b + 1) * T]
                    for qt in range(4):
                        ps_ctx = psB.tile([128, 512], f32, tag="ctx")
                        ps_sm = psB.tile([1, 512], f32, tag="sm")
                        # kt plan: full-width blocks then narrowed diagonals
                        # (kt, q-offset within the 512 q-window, width)
                        plan = [(kt, 0, 512) for kt in range(4 * qt)]
                        plan += [(4 * qt + r, 128 * r, 512 - 128 * r) for r in range(4)]

                        def issue_st(idx):
                            kt, off, w = plan[idx]
                            r = kt - 4 * qt
                            ps_st = psB.tile([128, 512], f32, tag="st", bufs=4, name="ps_st")
                            nc.tensor.matmul(ps_st[:, 0:w],
                                             KTb[b][:, kt * 128:(kt + 1) * 128],
                                             qt_full[:, qt * 512 + off: qt * 512 + off + w],
                                             start=True, stop=(r < 0))
                            if r >= 0:
                                # additive causal mask folded into PSUM on the PE
                                nc.tensor.matmul(ps_st[:, 0:w], ident[:],
                                                 amsk[:, r * 512 + off:(r + 1) * 512],
                                                 start=False, stop=True)
                            se = sxp.tile([128, 512], bf16, tag="se", name="se")
                            nc.scalar.activation(se[:, 0:w], ps_st[:, 0:w],
                                                 mybir.ActivationFunctionType.Exp,
                                                 scale=EXP_SCALE)
                            return se

                        n_plan = len(plan)
                        se_q = [issue_st(i) for i in range(min(3, n_plan))]
                        for idx in range(n_plan):
                            kt, off, w = plan[idx]
                            se_cur = se_q.pop(0)
                            if idx + 3 < n_plan:
                                se_q.append(issue_st(idx + 3))
                            last = (idx == n_plan - 1)
                            nc.tensor.matmul(ps_ctx[:, off:off + w],
                                             Vb[b][:, kt * 128:(kt + 1) * 128],
                                             se_cur[:, 0:w],
                                             start=(idx == 0), stop=last,
                                             skip_group_check=True)
                            nc.tensor.matmul(ps_sm[:, off:off + w], ones[:],
                                             se_cur[:, 0:w],
                                             start=(idx == 0), stop=last,
                                             skip_group_check=True)
                            cols_since_flush += 3 * w
                            if pending_norm is not None and cols_since_flush >= 5000:
                                flush_norm()
                        flush_norm()  # lead time ran out: flush before re-pending
                        pending_norm = (ps_ctx, ps_sm, h, b, qt)
                        cols_since_flush = 0
            flush_norm()
            sb.close()

            # ---------------- Phase C: output projection -----------------
            with ExitStack() as sc:
                obp = sc.enter_context(tc.tile_pool(name="obp", bufs=4))
                psC = sc.enter_context(tc.tile_pool(name="psC", bufs=4, space="PSUM"))
                for m in range(NT // 128):
                    for n in range(D // 512):
                        pso = psC.tile([128, 512], f32, tag="oc")
                        for h in range(HPC):
                            nc.tensor.matmul(pso[:], Ch[h][:, m * 128:(m + 1) * 128],
                                             wo_sb[:, h * D + n * 512: h * D + (n + 1) * 512],
                                             start=(h == 0), stop=(h == HPC - 1))
                        ob = obp.tile([128, 512], bf16, tag="ob")
                        nc.scalar.copy(ob[:], pso[:])
                        nc.sync.dma_start(outp[m * 128:(m + 1) * 128, n * 512:(n + 1) * 512], ob[:])

    nc.compile()
    return nc


def _get_nc():
    if "nc" not in _NC_CACHE:
        _NC_CACHE["nc"] = _build_program()
    return _NC_CACHE["nc"]


def _rope_tables():
    j = np.arange(0, DH, 2, dtype=np.float32) / np.float32(DH)
    inv_freq = (np.float32(1.0) / (np.float32(ROPE_BASE) ** j)).astype(np.float32)
    t = np.arange(T, dtype=np.float32)
    freqs = np.outer(t, inv_freq).astype(np.float32)   # (T, 64)
    c = np.cos(freqs).astype(np.float32).T             # (64, T)
    s = np.sin(freqs).astype(np.float32).T
    A = np.vstack([c, c]).astype(np.float32)           # (128, T)
    Bt = np.vstack([-s, s]).astype(np.float32)
    return np.ascontiguousarray(A).astype(BF), np.ascontiguousarray(Bt).astype(BF)


def _causal_masks():
    m = np.zeros((512, 512), dtype=np.float32)
    for r in range(4):
        p = np.arange(128)[:, None]
        f = np.arange(512)[None, :]
        m[r * 128:(r + 1) * 128, :] = np.where(r * 128 + p <= f, 0.0, MASK_NEG)
    return m.astype(BF)


def _make_in_maps(x, Wq, Wk, Wv, Wo):
    xT = np.ascontiguousarray(x.reshape(NT, D).T).astype(BF)
    A, Bt = _rope_tables()
    msk = _causal_masks()

    in_maps = []
    for g in range(8):
        in_maps.append({
            "xT": xT,
            "wq": np.ascontiguousarray(Wq[:, g * FPC:(g + 1) * FPC]).astype(BF),
            "wk": np.ascontiguousarray(Wk[:, g * DH:(g + 1) * DH]).astype(BF),
            "wv": np.ascontiguousarray(Wv[:, g * DH:(g + 1) * DH]).astype(BF),
            "wo": np.ascontiguousarray(Wo[g * FPC:(g + 1) * FPC, :]).astype(BF),
            "ropeA": A,
            "ropeB": Bt,
            "masks": msk,
        })
    return in_maps


def kernel(x, Wq, Wk, Wv, Wo):
    x = np.asarray(x, dtype=np.float32)
    Wq = np.asarray(Wq, dtype=np.float32)
    Wk = np.asarray(Wk, dtype=np.float32)
    Wv = np.asarray(Wv, dtype=np.float32)
    Wo = np.asarray(Wo, dtype=np.float32)

    nc = _get_nc()
    in_maps = _make_in_maps(x, Wq, Wk, Wv, Wo)

    res = run_bass_kernel_spmd(nc, in_maps, list(range(8)))
    acc = res.results[0]["outp"].astype(np.float32)
    for g in range(1, 8):
        acc = acc + res.results[g]["outp"].astype(np.float32)
    return np.ascontiguousarray(acc.reshape(B, T, D), dtype=np.float32)


# revision 8
# speedup vs baseline: 1.3023x; 1.0008x over previous
"""GroupedQueryAttention TRN2 Bass kernel, sharded over 8 NeuronCores.

Problem (hardcoded): B=2, T=2048, D=4096, 32 Q heads x 128, 8 KV groups x 128,
RoPE (base 5e5), causal, out = ctx @ Wo.

Sharding: core g owns Q heads 4g..4g+3 (Wq columns 512g:512g+512), KV group g
(Wk/Wv columns 128g:128g+128), and Wo rows 512g:512g+512 (row-parallel).
Each core computes a full-shape partial output; host sums the 8 partials.

v3: all-bf16 data path (f32 PSUM accumulation), fully SBUF-resident
intermediates, causal narrowing of diagonal score blocks, additive causal mask
applied in PSUM via an identity matmul (keeps the score->exp->ctx chain off
the DVE), depth-3 exp prefetch, consolidated slab DMAs, fast-reciprocal
softmax normalization off the PE critical path.
"""
import sys
import numpy as np

for _p in ("/opt/trn_rl_repo", "/root/.axon_site", "/root/.axon_site/_ro/trn_rl_repo"):
    if _p not in sys.path:
        sys.path.append(_p)

from contextlib import ExitStack

import ml_dtypes

import concourse.bass as bass
import concourse.tile as tile
from concourse import bacc, mybir
from concourse.bass_utils import run_bass_kernel_spmd
from concourse.masks import make_identity

B, T, D = 2, 2048, 4096
NH, NKV, DH = 32, 8, 128
HPC = NH // 8          # 4 q heads per core
FPC = HPC * DH         # 512 q features per core
ROPE_BASE = 500000.0
NT = B * T             # 4096 tokens
f32 = mybir.dt.float32
bf16 = mybir.dt.bfloat16
EXP_SCALE = 1.0 / float(np.sqrt(DH))
MASK_NEG = -20000.0
BF = ml_dtypes.bfloat16

_NC_CACHE = {}


def _build_program():
    nc = bacc.Bacc("TRN2", target_bir_lowering=False, debug=False)

    xT = nc.dram_tensor("xT", [D, NT], bf16, kind="ExternalInput").ap()
    wq = nc.dram_tensor("wq", [D, FPC], bf16, kind="ExternalInput").ap()
    wk = nc.dram_tensor("wk", [D, DH], bf16, kind="ExternalInput").ap()
    wv = nc.dram_tensor("wv", [D, DH], bf16, kind="ExternalInput").ap()
    wo = nc.dram_tensor("wo", [FPC, D], bf16, kind="ExternalInput").ap()
    ropeA = nc.dram_tensor("ropeA", [128, T], bf16, kind="ExternalInput").ap()
    ropeB = nc.dram_tensor("ropeB", [128, T], bf16, kind="ExternalInput").ap()
    masks = nc.dram_tensor("masks", [512, 512], bf16, kind="ExternalInput").ap()
    outp = nc.dram_tensor("outp", [NT, D], bf16, kind="ExternalOutput").ap()

    KC = D // 128  # 32 contraction chunks
    SLAB = 4
    NSLAB = KC // SLAB

    with tile.TileContext(nc) as tc, ExitStack() as s0:
        kvp = s0.enter_context(tc.tile_pool(name="kv", bufs=1))
        # persistent SBUF-resident intermediates
        Qh = [kvp.tile([128, NT], bf16, tag=f"Q{h}", name=f"Q{h}") for h in range(HPC)]
        KTb = [kvp.tile([128, T], bf16, tag=f"KT{i}", name=f"KT{i}") for i in range(B)]
        Vb = [kvp.tile([128, T], bf16, tag=f"V{i}", name=f"V{i}") for i in range(B)]
        Ch = [kvp.tile([128, NT], bf16, tag=f"C{h}", name=f"C{h}") for h in range(HPC)]
        tabA = kvp.tile([128, T], bf16, tag="tabA")
        tabB = kvp.tile([128, T], bf16, tag="tabB")
        amsk = kvp.tile([128, 4 * 512], bf16, tag="amsk")

        with ExitStack() as sa:
            wp = sa.enter_context(tc.tile_pool(name="wts", bufs=1))
            wq_sb = wp.tile([128, KC * FPC], bf16, tag="wq")
            wk_sb = wp.tile([128, KC * DH], bf16, tag="wk")
            wv_sb = wp.tile([128, KC * DH], bf16, tag="wv")
            xsp = sa.enter_context(tc.tile_pool(name="xs", bufs=3))
            evp = sa.enter_context(tc.tile_pool(name="ev", bufs=1))
            psA = sa.enter_context(tc.tile_pool(name="psA", bufs=1, space="PSUM"))

            def load_w_slab(s):
                # one consolidated DMA per weight tensor per 4-chunk slab
                nc.sync.dma_start(
                    wq_sb[:, s * SLAB * FPC:(s + 1) * SLAB * FPC]
                    .rearrange("p (k c) -> p k c", k=SLAB),
                    wq[s * 512:(s + 1) * 512, :]
                    .rearrange("(k p) c -> p k c", k=SLAB))
                nc.sync.dma_start(
                    wk_sb[:, s * SLAB * DH:(s + 1) * SLAB * DH]
                    .rearrange("p (k c) -> p k c", k=SLAB),
                    wk[s * 512:(s + 1) * 512, :]
                    .rearrange("(k p) c -> p k c", k=SLAB))
                nc.sync.dma_start(
                    wv_sb[:, s * SLAB * DH:(s + 1) * SLAB * DH]
                    .rearrange("p (k c) -> p k c", k=SLAB),
                    wv[s * 512:(s + 1) * 512, :]
                    .rearrange("(k p) c -> p k c", k=SLAB))

            def load_xsl(n, s):
                xsl = xsp.tile([128, SLAB * 512], bf16, tag="xs", name="xsl")
                nc.sync.dma_start(
                    xsl[:].rearrange("p (k c) -> p k c", k=SLAB),
                    xT[s * 512:(s + 1) * 512, n * 512:(n + 1) * 512]
                    .rearrange("(k p) c -> p k c", k=SLAB))
                return xsl

            # critical-path first: interleave slab-0 weight and x chunks at
            # 128-row granularity so the first matmul waits on ~256KB, not 1.25MB
            xsl0 = xsp.tile([128, SLAB * 512], bf16, tag="xs", name="xsl")
            for k in range(SLAB):
                nc.sync.dma_start(wq_sb[:, k * FPC:(k + 1) * FPC],
                                  wq[k * 128:(k + 1) * 128, :])
                nc.sync.dma_start(xsl0[:, k * 512:(k + 1) * 512],
                                  xT[k * 128:(k + 1) * 128, 0:512])
                nc.sync.dma_start(wk_sb[:, k * DH:(k + 1) * DH],
                                  wk[k * 128:(k + 1) * 128, :])
                nc.sync.dma_start(wv_sb[:, k * DH:(k + 1) * DH],
                                  wv[k * 128:(k + 1) * 128, :])
            load_w_slab(1)

            # setup that doesn't gate the first matmuls
            ident_f = kvp.tile([128, 128], f32, tag="ident_f")
            make_identity(nc, ident_f[:])
            ident = kvp.tile([128, 128], bf16, tag="ident")
            nc.vector.tensor_copy(ident[:], ident_f[:])
            ones_f = kvp.tile([128, 1], f32, tag="ones_f")
            nc.vector.memset(ones_f[:], 1.0)
            ones = kvp.tile([128, 1], bf16, tag="ones")
            nc.vector.tensor_copy(ones[:], ones_f[:])
            ones_row_f = kvp.tile([1, 128], f32, tag="ones_row_f")
            nc.vector.memset(ones_row_f[:], 1.0)
            ones_row = kvp.tile([1, 128], bf16, tag="ones_row")
            nc.vector.tensor_copy(ones_row[:], ones_row_f[:])

            # ---------------- Phase A: projections + RoPE -----------------
            def stationary(m, k):
                if m < HPC:
                    return wq_sb[:, k * FPC + m * 128: k * FPC + (m + 1) * 128]
                if m == HPC:
                    return wk_sb[:, k * DH:(k + 1) * DH]
                return wv_sb[:, k * DH:(k + 1) * DH]

            pending_vt = None

            def flush_vt():
                nonlocal pending_vt
                if pending_vt is None:
                    return
                vt_p, n_p = pending_vt
                b_p = n_p // 4
                for i in range(4):
                    ptr = psA.tile([128, 128], bf16, tag="tr", bufs=2, name="ptr")
                    nc.tensor.transpose(ptr[:], vt_p[:, i * 128:(i + 1) * 128], ident[:])
                    c_local = 4 * (n_p % 4) + i
                    nc.scalar.copy(Vb[b_p][:, c_local * 128:c_local * 128 + 128], ptr[:])
                pending_vt = None

            for n in range(NT // 512):
                b, tloc = n // 4, 512 * (n % 4)
                ps = [psA.tile([128, 512], f32, tag=f"ps{m}", name=f"ps{m}")
                      for m in range(6)]
                for s in range(NSLAB):
                    xsl = xsl0 if (n == 0 and s == 0) else load_xsl(n, s)
                    if n == 0 and s + 2 < NSLAB:
                        load_w_slab(s + 2)
                    for m in range(6):
                        for j in range(SLAB):
                            k = s * SLAB + j
                            nc.tensor.matmul(ps[m][:], stationary(m, k),
                                             xsl[:, j * 512:(j + 1) * 512],
                                             start=(k == 0), stop=(k == KC - 1))
                    if s == 0:
                        flush_vt()   # prev n-tile's V transposes, PE already warm here
                if n == 0:
                    nc.sync.dma_start(tabA[:], ropeA)
                    nc.sync.dma_start(tabB[:], ropeB)
                # evict: ACT copies first so PSUM banks free at ACT pace
                qes = []
                for m in range(5):
                    qe = evp.tile([128, 512], bf16, tag="qe", bufs=6, name=f"qe{m}")
                    nc.scalar.copy(qe[:], ps[m][:])
                    qes.append(qe)
                vt = evp.tile([128, 512], bf16, tag="vt", bufs=2, name="vt")
                nc.scalar.copy(vt[:], ps[5][:])
                pending_vt = (vt, n)
                # rope chains on DVE (bf16, 2x rate)
                tA = tabA[:, tloc:tloc + 512]
                tB = tabB[:, tloc:tloc + 512]
                for m in range(5):
                    qe = qes[m]
                    sw = evp.tile([128, 512], bf16, tag="sw", bufs=1, name="sw")
                    nc.vector.tensor_copy(sw[0:64, :], qe[64:128, :])
                    nc.vector.tensor_copy(sw[64:128, :], qe[0:64, :])
                    mm = evp.tile([128, 512], bf16, tag="mm", bufs=1, name="mm")
                    nc.vector.tensor_mul(mm[:], sw[:], tB)
                    tt = evp.tile([128, 512], bf16, tag="tt", bufs=1, name="tt")
                    nc.vector.tensor_mul(tt[:], qe[:], tA)
                    if m < HPC:
                        nc.vector.tensor_add(Qh[m][:, n * 512:(n + 1) * 512], tt[:], mm[:])
                    else:
                        nc.vector.tensor_add(KTb[b][:, tloc:tloc + 512], tt[:], mm[:])
            flush_vt()

        # ---------------- Phase B: attention -----------------
        with ExitStack() as sbc:
            wop = sbc.enter_context(tc.tile_pool(name="wop", bufs=1))
            wo_sb = wop.tile([128, HPC * D], bf16, tag="wo")
            for r in range(4):
                nc.sync.dma_start(amsk[:, r * 512:(r + 1) * 512],
                                  masks[r * 128:(r + 1) * 128, :])
            for h in range(HPC):
                nc.sync.dma_start(wo_sb[:, h * D:(h + 1) * D], wo[h * 128:(h + 1) * 128, :])

            sb = sbc.enter_context(ExitStack())
            sxp = sb.enter_context(tc.tile_pool(name="sxp", bufs=8))
            smp = sb.enter_context(tc.tile_pool(name="smp", bufs=2))
            psB = sb.enter_context(tc.tile_pool(name="psB", bufs=2, space="PSUM"))

            pending_norm = None
            cols_since_flush = 0

            def flush_norm():
                nonlocal pending_norm
                if pending_norm is None:
                    return
                ps_ctx_p, ps_sm_p, h_p, b_p, qt_p = pending_norm
                rs = smp.tile([1, 512], f32, tag="rs", name="rs")
                nc.vector.reciprocal_approx_fast(out=rs[:], in_=ps_sm_p[:])
                rs_b = smp.tile([1, 512], bf16, tag="rsb", name="rsb")
                nc.vector.tensor_copy(rs_b[:], rs[:])
                # broadcast recip across partitions: ones_row^T @ rs_b (K=1 matmul)
                ps_bc = psB.tile([128, 512], f32, tag="st", bufs=4, name="ps_bc")
                nc.tensor.matmul(ps_bc[:], ones_row[:], rs_b[:], start=True, stop=True)
                bcs = smp.tile([128, 512], bf16, tag="bcs", name="bcs")
                nc.vector.tensor_copy(bcs[:], ps_bc[:])
                nc.vector.tensor_mul(
                    Ch[h_p][:, b_p * T + qt_p * 512: b_p * T + (qt_p + 1) * 512],
                    ps_ctx_p[:], bcs[:])
                pending_norm = None

            for b in range(B):
                for h in range(HPC):
                    qt_full = Qh[h][:, b * T:(b + 1) * T]
                    for qt in range(4):
                        ps_ctx = psB.tile([128, 512], f32, tag="ctx")
                        ps_sm = psB.tile([1, 512], f32, tag="sm")
                        # kt plan: full-width blocks then narrowed diagonals
                        # (kt, q-offset within the 512 q-window, width)
                        plan = [(kt, 0, 512) for kt in range(4 * qt)]
                        plan += [(4 * qt + r, 128 * r, 512 - 128 * r) for r in range(4)]

                        def issue_st(idx):
                            kt, off, w = plan[idx]
                            r = kt - 4 * qt
                            ps_st = psB.tile([128, 512], f32, tag="st", bufs=4, name="ps_st")
                            nc.tensor.matmul(ps_st[:, 0:w],
                                             KTb[b][:, kt * 128:(kt + 1) * 128],
                                             qt_full[:, qt * 512 + off: qt * 512 + off + w],
                                             start=True, stop=(r < 0))
                            if r >= 0:
                                # additive causal mask folded into PSUM on the PE
                                nc.tensor.matmul(ps_st[:, 0:w], ident[:],
                                                 amsk[:, r * 512 + off:(r + 1) * 512],
                                                 start=False, stop=True)
                            se = sxp.tile([128, 512], bf16, tag="se", name="se")
                            nc.scalar.activation(se[:, 0:w], ps_st[:, 0:w],
                                                 mybir.ActivationFunctionType.Exp,
                                                 scale=EXP_SCALE)
                            return se

                        n_plan = len(plan)
                        se_q = [issue_st(i) for i in range(min(2, n_plan))]
                        for idx in range(n_plan):
                            kt, off, w = plan[idx]
                            se_cur = se_q.pop(0)
                            if idx + 2 < n_plan:
                                se_q.append(issue_st(idx + 2))
                            last = (idx == n_plan - 1)
                            nc.tensor.matmul(ps_ctx[:, off:off + w],
                                             Vb[b][:, kt * 128:(kt + 1) * 128],
                                             se_cur[:, 0:w],
                                             start=(idx == 0), stop=last,
                                             skip_group_check=True)
                            nc.tensor.matmul(ps_sm[:, off:off + w], ones[:],
                                             se_cur[:, 0:w],
                                             start=(idx == 0), stop=last,
                                             skip_group_check=True)
                            cols_since_flush += 3 * w
                            if pending_norm is not None and cols_since_flush >= 5000:
                                flush_norm()
                        flush_norm()  # lead time ran out: flush before re-pending
                        pending_norm = (ps_ctx, ps_sm, h, b, qt)
                        cols_since_flush = 0
            flush_norm()
            sb.close()

            # ---------------- Phase C: output projection -----------------
            with ExitStack() as sc:
                obp = sc.enter_context(tc.tile_pool(name="obp", bufs=4))
                psC = sc.enter_context(tc.tile_pool(name="psC", bufs=4, space="PSUM"))
                for m in range(NT // 128):
                    for n in range(D // 512):
                        pso = psC.tile([128, 512], f32, tag="oc")
                        for h in range(HPC):
                            nc.tensor.matmul(pso[:], Ch[h][:, m * 128:(m + 1) * 128],
                                             wo_sb[:, h * D + n * 512: h * D + (n + 1) * 512],
                                             start=(h == 0), stop=(h == HPC - 1))
                        ob = obp.tile([128, 512], bf16, tag="ob")
                        nc.scalar.copy(ob[:], pso[:])
                        nc.sync.dma_start(outp[m * 128:(m + 1) * 128, n * 512:(n + 1) * 512], ob[:])

    nc.compile()
    return nc


def _get_nc():
    if "nc" not in _NC_CACHE:
        _NC_CACHE["nc"] = _build_program()
    return _NC_CACHE["nc"]


def _rope_tables():
    j = np.arange(0, DH, 2, dtype=np.float32) / np.float32(DH)
    inv_freq = (np.float32(1.0) / (np.float32(ROPE_BASE) ** j)).astype(np.float32)
    t = np.arange(T, dtype=np.float32)
    freqs = np.outer(t, inv_freq).astype(np.float32)   # (T, 64)
    c = np.cos(freqs).astype(np.float32).T             # (64, T)
    s = np.sin(freqs).astype(np.float32).T
    A = np.vstack([c, c]).astype(np.float32)           # (128, T)
    Bt = np.vstack([-s, s]).astype(np.float32)
    return np.ascontiguousarray(A).astype(BF), np.ascontiguousarray(Bt).astype(BF)


def _causal_masks():
    m = np.zeros((512, 512), dtype=np.float32)
    for r in range(4):
        p = np.arange(128)[:, None]
        f = np.arange(512)[None, :]
        m[r * 128:(r + 1) * 128, :] = np.where(r * 128 + p <= f, 0.0, MASK_NEG)
    return m.astype(BF)


def _make_in_maps(x, Wq, Wk, Wv, Wo):
    xT = np.ascontiguousarray(x.reshape(NT, D).T).astype(BF)
    A, Bt = _rope_tables()
    msk = _causal_masks()

    in_maps = []
    for g in range(8):
        in_maps.append({
            "xT": xT,
            "wq": np.ascontiguousarray(Wq[:, g * FPC:(g + 1) * FPC]).astype(BF),
            "wk": np.ascontiguousarray(Wk[:, g * DH:(g + 1) * DH]).astype(BF),
            "wv": np.ascontiguousarray(Wv[:, g * DH:(g + 1) * DH]).astype(BF),
            "wo": np.ascontiguousarray(Wo[g * FPC:(g + 1) * FPC, :]).astype(BF),
            "ropeA": A,
            "ropeB": Bt,
            "masks": msk,
        })
    return in_maps


def kernel(x, Wq, Wk, Wv, Wo):
    x = np.asarray(x, dtype=np.float32)
    Wq = np.asarray(Wq, dtype=np.float32)
    Wk = np.asarray(Wk, dtype=np.float32)
    Wv = np.asarray(Wv, dtype=np.float32)
    Wo = np.asarray(Wo, dtype=np.float32)

    nc = _get_nc()
    in_maps = _make_in_maps(x, Wq, Wk, Wv, Wo)

    res = run_bass_kernel_spmd(nc, in_maps, list(range(8)))
    acc = res.results[0]["outp"].astype(np.float32)
    for g in range(1, 8):
        acc = acc + res.results[g]["outp"].astype(np.float32)
    return np.ascontiguousarray(acc.reshape(B, T, D), dtype=np.float32)
